# revision 9
# baseline (speedup 1.0000x reference)
"""Trainium2 Bass kernel for nn_Attn_40046275068166.

Tiny causal MHA over huge batch: x[B=65536, T=34, D=6], 2 heads, head_dim 3.
Strategy: pure data parallelism over 8 cores (batch sharded), batch on the
128 SBUF partitions inside each core. All per-example compute is expressed
as DVE tensor ops with broadcast access patterns; exp runs on the scalar
engine (ACT). Software-pipelined: phase A (projections + scores) of tile n
overlaps ACT exp of tile n-1 and phase B (softmax-normalize + PV + output
projection) of tile n-1. Raw bass (no Tile framework) with explicit
semaphores — this walrus build allows at most one sync-wait per instruction,
so every multi-dependency is expressed as standalone wait ops.

Math identity used to skip separate q/k projections:
  s[b,h,i,j] = q_i . k_j / sqrt(hd) = xp_i^T A_h xp_j,  A_h = Wq_h^T Wk_h/sqrt(hd)
so only y = A_h xp (per j) and v = Wv xt are projected, and s = xp_i . y_j.
Causal mask applied additively (-1e9) before exp.
"""

import math
from contextlib import ExitStack
from functools import lru_cache

import numpy as np

import concourse.bass as bass
from concourse import mybir
from concourse.bass_utils import run_bass_kernel_spmd

NCORES = 8
T = 34
D = 6
NH = 2
HD = 3
POS = 3
TT = T * T          # 1156
STT = NH * TT       # 2312 score elems per example
P = 128

F32 = mybir.dt.float32

# constants vector layout (element offsets)
OFF_A2 = 0          # [2][6][3]  w=0: y-proj weights, w=1: v-proj weights
OFF_WO = 36         # [6][6]     WoM[dm][e]
OFF_MASK = 72       # [1156]     additive causal mask (0 / -1e9)
CLEN = 72 + TT


def _ap(t, off, dims):
    """AP on SBUF tensor t: explicit free dims [(stride, count), ...]."""
    p0 = t[:].ap[0]
    return bass.AP(tensor=t, offset=off, ap=[list(p0)] + [list(d) for d in dims])


def build_kernel(bc, G):
    """bc: per-core batch, G: b-groups of 128 per pipeline tile."""
    assert bc % (P * G) == 0
    NT = bc // (P * G)
    GT = G * T * D          # x elements per partition per set (g,t,d)
    SC = G * STT            # score elems per partition per set

    nc = bass.Bass("TRN2")
    x = nc.dram_tensor("x", [bc, T, D], F32, kind="ExternalInput")
    wts = nc.dram_tensor("wts", [CLEN], F32, kind="ExternalInput")
    out = nc.dram_tensor("out", [bc, T, D], F32, kind="ExternalOutput")

    xr = x[:].rearrange("(n g p) t d -> n p g t d", g=G, p=P)
    outr = out[:].rearrange("(n g p) t d -> n p g t d", g=G, p=P)
    wts_b = bass.AP(tensor=wts, offset=0, ap=[[0, P], [1, CLEN]])

    with ExitStack() as ctx:
        sb = lambda nm, shape: ctx.enter_context(nc.sbuf_tensor(nm, shape, F32))
        wsb = sb("wsb", [P, CLEN])
        xin = sb("xin", [P, 2, G, T, D])
        yv = sb("yv", [P, 2, 2, G, T, D])     # [set][w][g][j][hc]
        t0 = sb("t0", [P, 2, G, NH, T, T])
        t1 = sb("t1", [P, 2, G, NH, T, T])
        tmp = sb("tmp", [P, T, D])
        den = sb("den", [P, G, NH, T])
        rcp = sb("rcp", [P, G, NH, T])
        o2 = sb("o2", [P, G, T, D])           # [g][t][e=(h,c)]
        prod = sb("prod", [P, G, D, T, D])    # [g][dm][t][e]
        res = sb("res", [P, 2, G, T, D])

        # dma_in/out_sem are parity-split: a DMA's 16 per-engine +1s only
        # certify completion if no OTHER DMA on the same semaphore is in
        # flight (8 engines finishing two DMAs also reads as "16"). With
        # even/odd semaphores plus the xin_done/res_done gating, at most one
        # DMA per semaphore is outstanding when a wait on it passes.
        sem_names = ["dma_in0", "dma_in1", "const", "xin_done", "s_done",
                     "e_done", "b_done", "res_done", "out0", "out1"]
        sems = {k: ctx.enter_context(nc.semaphore(name=k)) for k in sem_names}

        # element strides within a partition
        XIN_SET = G * T * D
        XIN_G = T * D
        YV_SET = 2 * G * T * D
        YV_W = G * T * D
        YV_G = T * D
        TS_SET = G * NH * TT            # t0/t1 set stride
        TS_G = NH * TT
        TS_H = TT

        block = ctx.enter_context(nc.Block())

        @block.gpsimd
        def _(sync):
            # SWDGE (software DGE): exactly one +16 sem increment per
            # dma_start on completion. HWDGE (nc.sync) fans a DMA out over
            # several hardware queues, each of which bumps the semaphore by
            # 16 — a single >=16*(n+1) wait would then fire before the whole
            # transfer has landed.
            sync.dma_start(out=wsb[:], in_=wts_b).then_inc(sems["const"], 16)
            for n in range(NT):
                s = n % 2
                if n >= 2:
                    sync.wait_ge(sems["xin_done"], n - 1)
                sync.dma_start(
                    out=_ap(xin, s * XIN_SET, [(XIN_G, G), (1, T * D)]),
                    in_=xr[n],
                ).then_inc(sems["dma_in0" if s == 0 else "dma_in1"], 16)
                if n >= 1:
                    sp = (n - 1) % 2
                    sync.wait_ge(sems["res_done"], n)
                    sync.dma_start(
                        out=outr[n - 1],
                        in_=_ap(res, sp * XIN_SET, [(XIN_G, G), (1, T * D)]),
                    ).then_inc(sems["out0" if sp == 0 else "out1"], 16)
            sync.wait_ge(sems["res_done"], NT)
            sync.dma_start(
                out=outr[NT - 1],
                in_=_ap(res, ((NT - 1) % 2) * XIN_SET, [(XIN_G, G), (1, T * D)]),
            ).then_inc(sems["out0" if (NT - 1) % 2 == 0 else "out1"], 16)
            # quiesce: don't let the program end with the last store in flight
            sync.wait_ge(sems["out0"], 16 * ((NT + 1) // 2))
            sync.wait_ge(sems["out1"], 16 * (NT // 2))

        @block.scalar
        def _(scalar):
            for n in range(NT):
                s = n % 2
                if n >= 2:
                    scalar.wait_ge(sems["b_done"], n - 1)
                scalar.wait_ge(sems["s_done"], n + 1)
                scalar.activation(
                    out=_ap(t1, s * TS_SET, [(1, SC)]),
                    in_=_ap(t0, s * TS_SET, [(1, SC)]),
                    func=mybir.ActivationFunctionType.Exp,
                ).then_inc(sems["e_done"], 1)

        @block.vector
        def _(vector):
            vector.wait_ge(sems["const"], 16)

            def phase_a(n):
                s = n % 2
                vector.wait_ge(sems["dma_in0" if s == 0 else "dma_in1"],
                               16 * (n // 2 + 1))
                # projections: yv[s, w, g, :, :] = sum_b x[.., pslice+b] * A2[w, :, b]
                for w in range(2):
                    for g in range(G):
                        xoff = s * XIN_SET + g * XIN_G + (3 - 3 * w)
                        yoff = s * YV_SET + w * YV_W + g * YV_G
                        for b in range(POS):
                            i0 = _ap(xin, xoff + b, [(D, T), (0, D)])
                            i1 = _ap(wsb, OFF_A2 + w * 18 + b, [(0, T), (3, D)])
                            if b == 0:
                                vector.tensor_mul(
                                    out=_ap(yv, yoff, [(D, T), (1, D)]),
                                    in0=i0, in1=i1)
                            else:
                                vector.tensor_mul(
                                    out=_ap(tmp, 0, [(D, T), (1, D)]),
                                    in0=i0, in1=i1)
                                vector.tensor_add(
                                    out=_ap(yv, yoff, [(D, T), (1, D)]),
                                    in0=_ap(yv, yoff, [(D, T), (1, D)]),
                                    in1=_ap(tmp, 0, [(D, T), (1, D)]))
                # scores: t = sum_a xp[i,a] * y[j,(h,a)]
                for a in range(POS):
                    dst = t0 if a == 0 else t1
                    for g in range(G):
                        for h in range(NH):
                            toff = s * TS_SET + g * TS_G + h * TS_H
                            mm = vector.tensor_mul(
                                out=_ap(dst, toff, [(T, T), (1, T)]),
                                in0=_ap(xin, s * XIN_SET + g * XIN_G + 3 + a,
                                        [(D, T), (0, T)]),
                                in1=_ap(yv, s * YV_SET + g * YV_G + h * HD + a,
                                        [(0, T), (D, T)]))
                            if a == POS - 1 and g == G - 1 and h == NH - 1:
                                mm.then_inc(sems["xin_done"], 1)
                    if a == 1:
                        vector.tensor_add(
                            out=_ap(t0, s * TS_SET, [(1, SC)]),
                            in0=_ap(t0, s * TS_SET, [(1, SC)]),
                            in1=_ap(t1, s * TS_SET, [(1, SC)]))
                # t1 += mask ; t0 += t1
                vector.tensor_add(
                    out=_ap(t1, s * TS_SET, [(TT, G * NH), (1, TT)]),
                    in0=_ap(t1, s * TS_SET, [(TT, G * NH), (1, TT)]),
                    in1=_ap(wsb, OFF_MASK, [(0, G * NH), (1, TT)]))
                vector.tensor_add(
                    out=_ap(t0, s * TS_SET, [(1, SC)]),
                    in0=_ap(t0, s * TS_SET, [(1, SC)]),
                    in1=_ap(t1, s * TS_SET, [(1, SC)])
                ).then_inc(sems["s_done"], 1)

            def phase_b(n):
                s = n % 2
                vector.wait_ge(sems["e_done"], n + 1)
                if n >= 2:
                    # WAR: res[s] still being read by out-DMA(n-2) (same parity)
                    vector.wait_ge(sems["out0" if s == 0 else "out1"],
                                   16 * (n // 2))
                # row sums over j, then reciprocal
                vector.tensor_reduce(
                    out=_ap(den, 0, [(1, G * NH * T)]),
                    in_=_ap(t1, s * TS_SET, [(T, G * NH * T), (1, T)]),
                    axis=mybir.AxisListType.X, op=mybir.AluOpType.add)
                vector.reciprocal(
                    out=_ap(rcp, 0, [(1, G * NH * T)]),
                    in_=_ap(den, 0, [(1, G * NH * T)]))
                # PV: o2[g, i, (h,c)] = sum_j e[g,h,i,j] * v[g,j,(h,c)]
                for c in range(HD):
                    for g in range(G):
                        for h in range(NH):
                            toff = s * TS_SET + g * TS_G + h * TS_H
                            mm = vector.tensor_mul(
                                out=_ap(t0, toff, [(T, T), (1, T)]),
                                in0=_ap(t1, toff, [(T, T), (1, T)]),
                                in1=_ap(yv, s * YV_SET + YV_W + g * YV_G + h * HD + c,
                                        [(0, T), (D, T)]))
                            if c == HD - 1 and g == G - 1 and h == NH - 1:
                                mm.then_inc(sems["b_done"], 1)
                    vector.tensor_reduce(
                        out=_ap(o2, c, [(T * D, G), (HD, NH), (D, T)]),
                        in_=_ap(t0, s * TS_SET, [(T, G * NH * T), (1, T)]),
                        axis=mybir.AxisListType.X, op=mybir.AluOpType.add)
                # normalize: o2 *= rcp (broadcast over c)
                for g in range(G):
                    vector.tensor_mul(
                        out=_ap(o2, g * T * D, [(D, T), (HD, NH), (1, HD)]),
                        in0=_ap(o2, g * T * D, [(D, T), (HD, NH), (1, HD)]),
                        in1=_ap(rcp, g * NH * T, [(1, T), (T, NH), (0, HD)]))
                # output projection: res[g,t,dm] = sum_e o2[g,t,e] * WoM[dm,e]
                for g in range(G):
                    for dm in range(D):
                        vector.tensor_mul(
                            out=_ap(prod, g * D * T * D + dm * T * D,
                                    [(D, T), (1, D)]),
                            in0=_ap(o2, g * T * D, [(D, T), (1, D)]),
                            in1=_ap(wsb, OFF_WO + dm * D, [(0, T), (1, D)]))
                vector.tensor_reduce(
                    out=_ap(res, s * XIN_SET, [(T * D, G), (1, D), (D, T)]),
                    in_=_ap(prod, 0, [(D, G * D * T), (1, D)]),
                    axis=mybir.AxisListType.X, op=mybir.AluOpType.add
                ).then_inc(sems["res_done"], 1)

            for n in range(NT):
                phase_a(n)
                if n >= 1:
                    phase_b(n - 1)
            phase_b(NT - 1)

    return nc


def _pack_weights(Wq, Wk, Wv, Wo):
    wts = np.zeros(CLEN, dtype=np.float32)
    scale = 1.0 / math.sqrt(HD)
    A2 = wts[OFF_A2:OFF_A2 + 36].reshape(2, D, POS)
    for h in range(NH):
        A2[0, h * HD:(h + 1) * HD, :] = (Wq[h * HD:(h + 1) * HD, :].T
                                         @ Wk[h * HD:(h + 1) * HD, :]) * scale
        A2[1, h * HD:(h + 1) * HD, :] = Wv[h * HD:(h + 1) * HD, :]
    wts[OFF_WO:OFF_WO + 36] = Wo.reshape(-1)
    mask = np.where(np.tril(np.ones((T, T))) > 0, 0.0, -1e9).astype(np.float32)
    wts[OFF_MASK:OFF_MASK + TT] = mask.reshape(-1)
    return wts


@lru_cache(maxsize=2)
def _cached_kernel(bc, G):
    return build_kernel(bc, G)


def kernel(x, Wq, Wk, Wv, Wo):
    x = np.ascontiguousarray(x, dtype=np.float32)
    B = x.shape[0]
    bc = B // NCORES
    G = 2
    nc = _cached_kernel(bc, G)
    wts = _pack_weights(np.asarray(Wq, dtype=np.float32),
                        np.asarray(Wk, dtype=np.float32),
                        np.asarray(Wv, dtype=np.float32),
                        np.asarray(Wo, dtype=np.float32))
    in_maps = [{"x": x[i * bc:(i + 1) * bc], "wts": wts} for i in range(NCORES)]
    r = run_bass_kernel_spmd(nc, in_maps, core_ids=list(range(NCORES)))
    return np.concatenate([m["out"] for m in r.results], axis=0)


# revision 13
# speedup vs baseline: 1.2316x; 1.2316x over previous
"""Trainium2 Bass kernel for nn_Attn_40046275068166.

Tiny causal MHA over huge batch: x[B=65536, T=34, D=6], 2 heads, head_dim 3.
Strategy: pure data parallelism over 8 cores (batch sharded), batch on the
128 SBUF partitions inside each core. All per-example compute is expressed
as DVE tensor ops with broadcast access patterns; exp runs on the scalar
engine (ACT). Software-pipelined: phase A (projections + scores) of tile n
overlaps ACT exp of tile n-1 and phase B (softmax-normalize + PV + output
projection) of tile n-1. Raw bass (no Tile framework) with explicit
semaphores — this walrus build allows at most one sync-wait per instruction,
so every multi-dependency is expressed as standalone wait ops.

Math identity used to skip separate q/k projections:
  s[b,h,i,j] = q_i . k_j / sqrt(hd) = xp_i^T A_h xp_j,  A_h = Wq_h^T Wk_h/sqrt(hd)
so only y = A_h xp (per j) and v = Wv xt are projected, and s = xp_i . y_j.
Causal mask applied additively (-1e9) before exp.
"""

import math
from contextlib import ExitStack
from functools import lru_cache

import numpy as np

import concourse.bass as bass
from concourse import mybir
from concourse.bass_utils import run_bass_kernel_spmd

NCORES = 8
T = 34
D = 6
NH = 2
HD = 3
POS = 3
TT = T * T          # 1156
STT = NH * TT       # 2312 score elems per example
P = 128

F32 = mybir.dt.float32

# constants vector layout (element offsets)
OFF_A2 = 0          # [2][6][3]  w=0: y-proj weights, w=1: v-proj weights
OFF_WO = 36         # [6][6]     WoM[dm][e]
OFF_MASK = 72       # [1156]     additive causal mask (0 / -1e9)
CLEN = 72 + TT


def _ap(t, off, dims):
    """AP on SBUF tensor t: explicit free dims [(stride, count), ...]."""
    p0 = t[:].ap[0]
    return bass.AP(tensor=t, offset=off, ap=[list(p0)] + [list(d) for d in dims])


def build_kernel(bc, G):
    """bc: per-core batch, G: b-groups of 128 per pipeline tile."""
    assert bc % (P * G) == 0
    NT = bc // (P * G)
    GT = G * T * D          # x elements per partition per set (g,t,d)
    SC = G * STT            # score elems per partition per set

    nc = bass.Bass("TRN2")
    x = nc.dram_tensor("x", [bc, T, D], F32, kind="ExternalInput")
    wts = nc.dram_tensor("wts", [CLEN], F32, kind="ExternalInput")
    out = nc.dram_tensor("out", [bc, T, D], F32, kind="ExternalOutput")

    xr = x[:].rearrange("(n g p) t d -> n p g t d", g=G, p=P)
    outr = out[:].rearrange("(n g p) t d -> n p g t d", g=G, p=P)
    wts_b = bass.AP(tensor=wts, offset=0, ap=[[0, P], [1, CLEN]])

    with ExitStack() as ctx:
        sb = lambda nm, shape: ctx.enter_context(nc.sbuf_tensor(nm, shape, F32))
        wsb = sb("wsb", [P, CLEN])
        xin = sb("xin", [P, 2, G, T, D])
        # [set][w][g][hc][j] — j innermost (stride 1): DVE broadcast reads
        # with non-unit inner strides cost ~1.7x; stride-0/1 run at full rate
        yv = sb("yv", [P, 2, 2, G, D, T])
        t0 = sb("t0", [P, 2, G, NH, T, T])
        t1 = sb("t1", [P, 2, G, NH, T, T])
        tmp = sb("tmp", [P, T, D])
        den = sb("den", [P, G, NH, T])
        rcp = sb("rcp", [P, G, NH, T])
        o2 = sb("o2", [P, G, T, D])           # [g][t][e=(h,c)]
        prod = sb("prod", [P, G, D, T, D])    # [g][dm][t][e]
        res = sb("res", [P, 2, G, T, D])

        # dma_in/out_sem are parity-split: a DMA's 16 per-engine +1s only
        # certify completion if no OTHER DMA on the same semaphore is in
        # flight (8 engines finishing two DMAs also reads as "16"). With
        # even/odd semaphores plus the xin_done/res_done gating, at most one
        # DMA per semaphore is outstanding when a wait on it passes.
        sem_names = ["dma_in0", "dma_in1", "const", "xin_done", "s_done",
                     "e_done", "b_done", "res_done", "out0", "out1"]
        sems = {k: ctx.enter_context(nc.semaphore(name=k)) for k in sem_names}

        # element strides within a partition
        XIN_SET = G * T * D
        XIN_G = T * D
        YV_SET = 2 * G * T * D
        YV_W = G * T * D
        YV_G = T * D
        TS_SET = G * NH * TT            # t0/t1 set stride
        TS_G = NH * TT
        TS_H = TT

        block = ctx.enter_context(nc.Block())

        @block.gpsimd
        def _(sync):
            # SWDGE (software DGE): exactly one +16 sem increment per
            # dma_start on completion. HWDGE (nc.sync) fans a DMA out over
            # several hardware queues, each of which bumps the semaphore by
            # 16 — a single >=16*(n+1) wait would then fire before the whole
            # transfer has landed.
            sync.dma_start(out=wsb[:], in_=wts_b).then_inc(sems["const"], 16)
            for n in range(NT):
                s = n % 2
                if n >= 2:
                    sync.wait_ge(sems["xin_done"], n - 1)
                sync.dma_start(
                    out=_ap(xin, s * XIN_SET, [(XIN_G, G), (1, T * D)]),
                    in_=xr[n],
                ).then_inc(sems["dma_in0" if s == 0 else "dma_in1"], 16)
                if n >= 1:
                    sp = (n - 1) % 2
                    sync.wait_ge(sems["res_done"], n)
                    sync.dma_start(
                        out=outr[n - 1],
                        in_=_ap(res, sp * XIN_SET, [(XIN_G, G), (1, T * D)]),
                    ).then_inc(sems["out0" if sp == 0 else "out1"], 16)
            sync.wait_ge(sems["res_done"], NT)
            sync.dma_start(
                out=outr[NT - 1],
                in_=_ap(res, ((NT - 1) % 2) * XIN_SET, [(XIN_G, G), (1, T * D)]),
            ).then_inc(sems["out0" if (NT - 1) % 2 == 0 else "out1"], 16)
            # quiesce: don't let the program end with the last store in flight
            sync.wait_ge(sems["out0"], 16 * ((NT + 1) // 2))
            sync.wait_ge(sems["out1"], 16 * (NT // 2))

        @block.scalar
        def _(scalar):
            for n in range(NT):
                s = n % 2
                if n >= 2:
                    scalar.wait_ge(sems["b_done"], n - 1)
                scalar.wait_ge(sems["s_done"], n + 1)
                scalar.activation(
                    out=_ap(t1, s * TS_SET, [(1, SC)]),
                    in_=_ap(t0, s * TS_SET, [(1, SC)]),
                    func=mybir.ActivationFunctionType.Exp,
                ).then_inc(sems["e_done"], 1)

        @block.vector
        def _(vector):
            vector.wait_ge(sems["const"], 16)

            def phase_a(n):
                s = n % 2
                vector.wait_ge(sems["dma_in0" if s == 0 else "dma_in1"],
                               16 * (n // 2 + 1))
                # projections: yv[s, w, g, :, :] = sum_b x[.., pslice+b] * A2[w, :, b]
                for w in range(2):
                    for g in range(G):
                        xoff = s * XIN_SET + g * XIN_G + (3 - 3 * w)
                        yoff = s * YV_SET + w * YV_W + g * YV_G
                        for b in range(POS):
                            # dims (hc, j): out yv[.., hc, j]
                            i0 = _ap(xin, xoff + b, [(0, D), (D, T)])
                            i1 = _ap(wsb, OFF_A2 + w * 18 + b, [(3, D), (0, T)])
                            if b == 0:
                                vector.tensor_mul(
                                    out=_ap(yv, yoff, [(T, D), (1, T)]),
                                    in0=i0, in1=i1)
                            else:
                                vector.tensor_mul(
                                    out=_ap(tmp, 0, [(T, D), (1, T)]),
                                    in0=i0, in1=i1)
                                vector.tensor_add(
                                    out=_ap(yv, yoff, [(1, T * D)]),
                                    in0=_ap(yv, yoff, [(1, T * D)]),
                                    in1=_ap(tmp, 0, [(1, T * D)]))
                # scores: t = sum_a xp[i,a] * y[j,(h,a)]
                for a in range(POS):
                    dst = t0 if a == 0 else t1
                    for g in range(G):
                        for h in range(NH):
                            toff = s * TS_SET + g * TS_G + h * TS_H
                            mm = vector.tensor_mul(
                                out=_ap(dst, toff, [(T, T), (1, T)]),
                                in0=_ap(xin, s * XIN_SET + g * XIN_G + 3 + a,
                                        [(D, T), (0, T)]),
                                in1=_ap(yv, s * YV_SET + g * YV_G
                                        + (h * HD + a) * T, [(0, T), (1, T)]))
                            if a == POS - 1 and g == G - 1 and h == NH - 1:
                                mm.then_inc(sems["xin_done"], 1)
                    if a == 1:
                        vector.tensor_add(
                            out=_ap(t0, s * TS_SET, [(1, SC)]),
                            in0=_ap(t0, s * TS_SET, [(1, SC)]),
                            in1=_ap(t1, s * TS_SET, [(1, SC)]))
                # t1 += mask ; t0 += t1
                vector.tensor_add(
                    out=_ap(t1, s * TS_SET, [(TT, G * NH), (1, TT)]),
                    in0=_ap(t1, s * TS_SET, [(TT, G * NH), (1, TT)]),
                    in1=_ap(wsb, OFF_MASK, [(0, G * NH), (1, TT)]))
                vector.tensor_add(
                    out=_ap(t0, s * TS_SET, [(1, SC)]),
                    in0=_ap(t0, s * TS_SET, [(1, SC)]),
                    in1=_ap(t1, s * TS_SET, [(1, SC)])
                ).then_inc(sems["s_done"], 1)

            def phase_b(n):
                s = n % 2
                vector.wait_ge(sems["e_done"], n + 1)
                if n >= 2:
                    # WAR: res[s] still being read by out-DMA(n-2) (same parity)
                    vector.wait_ge(sems["out0" if s == 0 else "out1"],
                                   16 * (n // 2))
                # row sums over j, then reciprocal
                vector.tensor_reduce(
                    out=_ap(den, 0, [(1, G * NH * T)]),
                    in_=_ap(t1, s * TS_SET, [(T, G * NH * T), (1, T)]),
                    axis=mybir.AxisListType.X, op=mybir.AluOpType.add)
                vector.reciprocal(
                    out=_ap(rcp, 0, [(1, G * NH * T)]),
                    in_=_ap(den, 0, [(1, G * NH * T)]))
                # PV: o2[g, i, (h,c)] = sum_j e[g,h,i,j] * v[g,j,(h,c)]
                for c in range(HD):
                    for g in range(G):
                        for h in range(NH):
                            toff = s * TS_SET + g * TS_G + h * TS_H
                            mm = vector.tensor_mul(
                                out=_ap(t0, toff, [(T, T), (1, T)]),
                                in0=_ap(t1, toff, [(T, T), (1, T)]),
                                in1=_ap(yv, s * YV_SET + YV_W + g * YV_G
                                        + (h * HD + c) * T, [(0, T), (1, T)]))
                            if c == HD - 1 and g == G - 1 and h == NH - 1:
                                mm.then_inc(sems["b_done"], 1)
                    vector.tensor_reduce(
                        out=_ap(o2, c, [(T * D, G), (HD, NH), (D, T)]),
                        in_=_ap(t0, s * TS_SET, [(T, G * NH * T), (1, T)]),
                        axis=mybir.AxisListType.X, op=mybir.AluOpType.add)
                # normalize: o2 *= rcp (broadcast over c)
                for g in range(G):
                    vector.tensor_mul(
                        out=_ap(o2, g * T * D, [(D, T), (HD, NH), (1, HD)]),
                        in0=_ap(o2, g * T * D, [(D, T), (HD, NH), (1, HD)]),
                        in1=_ap(rcp, g * NH * T, [(1, T), (T, NH), (0, HD)]))
                # output projection: res[g,t,dm] = sum_e o2[g,t,e] * WoM[dm,e]
                for g in range(G):
                    for dm in range(D):
                        vector.tensor_mul(
                            out=_ap(prod, g * D * T * D + dm * T * D,
                                    [(D, T), (1, D)]),
                            in0=_ap(o2, g * T * D, [(D, T), (1, D)]),
                            in1=_ap(wsb, OFF_WO + dm * D, [(0, T), (1, D)]))
                vector.tensor_reduce(
                    out=_ap(res, s * XIN_SET, [(T * D, G), (1, D), (D, T)]),
                    in_=_ap(prod, 0, [(D, G * D * T), (1, D)]),
                    axis=mybir.AxisListType.X, op=mybir.AluOpType.add
                ).then_inc(sems["res_done"], 1)

            for n in range(NT):
                phase_a(n)
                if n >= 1:
                    phase_b(n - 1)
            phase_b(NT - 1)

    return nc


def _pack_weights(Wq, Wk, Wv, Wo):
    wts = np.zeros(CLEN, dtype=np.float32)
    scale = 1.0 / math.sqrt(HD)
    A2 = wts[OFF_A2:OFF_A2 + 36].reshape(2, D, POS)
    for h in range(NH):
        A2[0, h * HD:(h + 1) * HD, :] = (Wq[h * HD:(h + 1) * HD, :].T
                                         @ Wk[h * HD:(h + 1) * HD, :]) * scale
        A2[1, h * HD:(h + 1) * HD, :] = Wv[h * HD:(h + 1) * HD, :]
    wts[OFF_WO:OFF_WO + 36] = Wo.reshape(-1)
    mask = np.where(np.tril(np.ones((T, T))) > 0, 0.0, -1e9).astype(np.float32)
    wts[OFF_MASK:OFF_MASK + TT] = mask.reshape(-1)
    return wts


@lru_cache(maxsize=2)
def _cached_kernel(bc, G):
    return build_kernel(bc, G)


def kernel(x, Wq, Wk, Wv, Wo):
    x = np.ascontiguousarray(x, dtype=np.float32)
    B = x.shape[0]
    bc = B // NCORES
    G = 2
    nc = _cached_kernel(bc, G)
    wts = _pack_weights(np.asarray(Wq, dtype=np.float32),
                        np.asarray(Wk, dtype=np.float32),
                        np.asarray(Wv, dtype=np.float32),
                        np.asarray(Wo, dtype=np.float32))
    in_maps = [{"x": x[i * bc:(i + 1) * bc], "wts": wts} for i in range(NCORES)]
    r = run_bass_kernel_spmd(nc, in_maps, core_ids=list(range(NCORES)))
    return np.concatenate([m["out"] for m in r.results], axis=0)


# revision 20
# speedup vs baseline: 1.5155x; 1.2305x over previous
"""Trainium2 Bass kernel for nn_Attn_40046275068166.

Tiny causal MHA over huge batch: x[B=65536, T=34, D=6], 2 heads, head_dim 3.
Strategy: pure data parallelism over 8 cores (batch sharded), batch on the
128 SBUF partitions inside each core. All per-example compute is expressed
as DVE tensor ops with broadcast access patterns; exp runs on the scalar
engine (ACT). Software-pipelined: phase A (projections + scores) of tile n
overlaps ACT exp of tile n-1 and phase B (softmax-normalize + PV + output
projection) of tile n-1. Raw bass (no Tile framework) with explicit
semaphores — this walrus build allows at most one sync-wait per instruction,
so every multi-dependency is expressed as standalone wait ops.

Math identity used to skip separate q/k projections:
  s[b,h,i,j] = q_i . k_j / sqrt(hd) = xp_i^T A_h xp_j,  A_h = Wq_h^T Wk_h/sqrt(hd)
so only y = A_h xp (per j) and v = Wv xt are projected, and s = xp_i . y_j.
Causal mask applied additively (-1e9) before exp.
"""

import math
from contextlib import ExitStack
from functools import lru_cache

import numpy as np

import concourse.bass as bass
from concourse import mybir
from concourse.bass_utils import run_bass_kernel_spmd

NCORES = 8
T = 34
D = 6
NH = 2
HD = 3
POS = 3
TT = T * T          # 1156
STT = NH * TT       # 2312 score elems per example
P = 128

F32 = mybir.dt.float32

# constants vector layout (element offsets)
OFF_A2 = 0          # [2][6][3]  w=0: y-proj weights, w=1: v-proj weights
OFF_WO = 36         # [6][6]     WoM[dm][e]
OFF_MASK = 72       # [1156]     additive causal mask (0 / -1e9)
CLEN = 72 + TT


def _ap(t, off, dims):
    """AP on SBUF tensor t: explicit free dims [(stride, count), ...]."""
    p0 = t[:].ap[0]
    return bass.AP(tensor=t, offset=off, ap=[list(p0)] + [list(d) for d in dims])


def build_kernel(bc, G):
    """bc: per-core batch, G: b-groups of 128 per pipeline tile."""
    assert bc % (P * G) == 0
    NT = bc // (P * G)
    GT = G * T * D          # x elements per partition per set (g,t,d)
    SC = G * STT            # score elems per partition per set

    nc = bass.Bass("TRN2")
    x = nc.dram_tensor("x", [bc, T, D], F32, kind="ExternalInput")
    wts = nc.dram_tensor("wts", [CLEN], F32, kind="ExternalInput")
    out = nc.dram_tensor("out", [bc, T, D], F32, kind="ExternalOutput")

    xr = x[:].rearrange("(n g p) t d -> n p g t d", g=G, p=P)
    outr = out[:].rearrange("(n g p) t d -> n p g t d", g=G, p=P)
    wts_b = bass.AP(tensor=wts, offset=0, ap=[[0, P], [1, CLEN]])

    with ExitStack() as ctx:
        sb = lambda nm, shape: ctx.enter_context(nc.sbuf_tensor(nm, shape, F32))
        wsb = sb("wsb", [P, CLEN])
        xin = sb("xin", [P, 2, G, T, D])
        # [set][w][g][hc][j] — j innermost (stride 1): DVE broadcast reads
        # with non-unit inner strides cost ~1.7x; stride-0/1 run at full rate
        yv = sb("yv", [P, 2, 2, G, D, T])
        pp = sb("pp", [P, G, NH, T, T])   # PV products (dead block stays 0)
        t0 = sb("t0", [P, 2, G, NH, T, T])
        t1 = sb("t1", [P, 2, G, NH, T, T])
        tmp = sb("tmp", [P, T, D])
        den = sb("den", [P, G, NH, T])
        rcp = sb("rcp", [P, G, NH, T])
        o2 = sb("o2", [P, G, T, D])           # [g][t][e=(h,c)]
        prod = sb("prod", [P, G, D, T, D])    # [g][dm][t][e]
        res = sb("res", [P, 2, G, T, D])

        # dma_in/out_sem are parity-split: a DMA's 16 per-engine +1s only
        # certify completion if no OTHER DMA on the same semaphore is in
        # flight (8 engines finishing two DMAs also reads as "16"). With
        # even/odd semaphores plus the xin_done/res_done gating, at most one
        # DMA per semaphore is outstanding when a wait on it passes.
        sem_names = ["dma_in0", "dma_in1", "const", "xin_done", "s_done",
                     "e_done", "b_done", "res_done", "out0", "out1"]
        sems = {k: ctx.enter_context(nc.semaphore(name=k)) for k in sem_names}

        # element strides within a partition
        XIN_SET = G * T * D
        XIN_G = T * D
        YV_SET = 2 * G * T * D
        YV_W = G * T * D
        YV_G = T * D
        TS_SET = G * NH * TT            # t0/t1 set stride
        TS_G = NH * TT
        TS_H = TT

        block = ctx.enter_context(nc.Block())

        @block.gpsimd
        def _(sync):
            # SWDGE (software DGE): exactly one +16 sem increment per
            # dma_start on completion. HWDGE (nc.sync) fans a DMA out over
            # several hardware queues, each of which bumps the semaphore by
            # 16 — a single >=16*(n+1) wait would then fire before the whole
            # transfer has landed.
            sync.dma_start(out=wsb[:], in_=wts_b).then_inc(sems["const"], 16)
            for n in range(NT):
                s = n % 2
                if n >= 2:
                    sync.wait_ge(sems["xin_done"], n - 1)
                sync.dma_start(
                    out=_ap(xin, s * XIN_SET, [(XIN_G, G), (1, T * D)]),
                    in_=xr[n],
                ).then_inc(sems["dma_in0" if s == 0 else "dma_in1"], 16)
                if n >= 1:
                    sp = (n - 1) % 2
                    sync.wait_ge(sems["res_done"], n)
                    sync.dma_start(
                        out=outr[n - 1],
                        in_=_ap(res, sp * XIN_SET, [(XIN_G, G), (1, T * D)]),
                    ).then_inc(sems["out0" if sp == 0 else "out1"], 16)
            sync.wait_ge(sems["res_done"], NT)
            sync.dma_start(
                out=outr[NT - 1],
                in_=_ap(res, ((NT - 1) % 2) * XIN_SET, [(XIN_G, G), (1, T * D)]),
            ).then_inc(sems["out0" if (NT - 1) % 2 == 0 else "out1"], 16)
            # quiesce: don't let the program end with the last store in flight
            sync.wait_ge(sems["out0"], 16 * ((NT + 1) // 2))
            sync.wait_ge(sems["out1"], 16 * (NT // 2))

        @block.scalar
        def _(scalar):
            for n in range(NT):
                s = n % 2
                if n >= 2:
                    scalar.wait_ge(sems["b_done"], n - 1)
                scalar.wait_ge(sems["s_done"], n + 1)
                scalar.activation(
                    out=_ap(t1, s * TS_SET, [(1, SC)]),
                    in_=_ap(t0, s * TS_SET, [(1, SC)]),
                    func=mybir.ActivationFunctionType.Exp,
                ).then_inc(sems["e_done"], 1)

        @block.vector
        def _(vector):
            vector.wait_ge(sems["const"], 16)
            # Causal blocks over the TxT score plane (H = T//2):
            #   blk A: i<H,  j<H   (has diagonal -> mask)
            #   blk B: i>=H, j<H   (fully causal -> no mask)
            #   blk C: i>=H, j>=H  (has diagonal -> mask)
            # dead:   i<H,  j>=H   (never computed)
            # t0's dead block is set to -1e9 once (exp -> 0); pp's dead block
            # to 0 once (reduce adds 0). Neither is ever rewritten.
            H = T // 2
            BLKS = [(0, 0), (H, 0), (H, H)]
            for s in range(2):
                vector.memset(
                    _ap(t0, s * TS_SET + H, [(TT, G * NH), (T, H), (1, T - H)]),
                    -1e9)
            vector.memset(_ap(pp, H, [(TT, G * NH), (T, H), (1, T - H)]), 0.0)

            def sadd(dst, dof, i0t, i0o, i1t, i1o):
                """dst[blocks] = in0 + in1 over AB (j<H) and C regions."""
                for (ro, li, lj) in ((0, T, H), (H * T + H, T - H, T - H)):
                    vector.tensor_add(
                        out=_ap(dst, dof + ro, [(TT, G * NH), (T, li), (1, lj)]),
                        in0=_ap(i0t, i0o + ro, [(TT, G * NH), (T, li), (1, lj)]),
                        in1=_ap(i1t, i1o + ro, [(TT, G * NH), (T, li), (1, lj)]))

            def phase_a(n):
                s = n % 2
                vector.wait_ge(sems["dma_in0" if s == 0 else "dma_in1"],
                               16 * (n // 2 + 1))
                # projections: yv[s, w, g, hc, j] = sum_b x[j, base+b]*A2[w,hc,b]
                for w in range(2):
                    for g in range(G):
                        xoff = s * XIN_SET + g * XIN_G + (3 - 3 * w)
                        yoff = s * YV_SET + w * YV_W + g * YV_G
                        for b in range(POS):
                            i0 = _ap(xin, xoff + b, [(0, D), (D, T)])
                            i1 = _ap(wsb, OFF_A2 + w * 18 + b, [(3, D), (0, T)])
                            if b == 0:
                                vector.tensor_mul(
                                    out=_ap(yv, yoff, [(T, D), (1, T)]),
                                    in0=i0, in1=i1)
                            else:
                                vector.tensor_mul(
                                    out=_ap(tmp, 0, [(T, D), (1, T)]),
                                    in0=i0, in1=i1)
                                vector.tensor_add(
                                    out=_ap(yv, yoff, [(1, T * D)]),
                                    in0=_ap(yv, yoff, [(1, T * D)]),
                                    in1=_ap(tmp, 0, [(1, T * D)]))
                # scores: t[g,h,i,j] = sum_a xp[g,i,a] * y[g,(h,a),j], blocked
                for a in range(POS):
                    dst = t0 if a == 0 else t1
                    for h in range(NH):
                        for bi, (i0b, j0b) in enumerate(BLKS):
                            li = H if i0b == 0 else T - H
                            lj = H if j0b == 0 else T - H
                            mm = vector.tensor_mul(
                                out=_ap(dst, s * TS_SET + h * TT + i0b * T + j0b,
                                        [(TS_G, G), (T, li), (1, lj)]),
                                in0=_ap(xin, s * XIN_SET + 3 + a + i0b * D,
                                        [(XIN_G, G), (D, li), (0, lj)]),
                                in1=_ap(yv, s * YV_SET + (h * HD + a) * T + j0b,
                                        [(YV_G, G), (0, li), (1, lj)]))
                            if a == POS - 1 and h == NH - 1 and bi == len(BLKS) - 1:
                                mm.then_inc(sems["xin_done"], 1)
                    if a == 1:
                        sadd(t0, s * TS_SET, t0, s * TS_SET, t1, s * TS_SET)
                # t1 += mask on diagonal blocks A and C; then t0 += t1
                for ro in (0, H * T + H):
                    vector.tensor_add(
                        out=_ap(t1, s * TS_SET + ro,
                                [(TT, G * NH), (T, H), (1, H)]),
                        in0=_ap(t1, s * TS_SET + ro,
                                [(TT, G * NH), (T, H), (1, H)]),
                        in1=_ap(wsb, OFF_MASK + ro,
                                [(0, G * NH), (T, H), (1, H)]))
                vector.tensor_add(
                    out=_ap(t0, s * TS_SET, [(TT, G * NH), (T, T), (1, H)]),
                    in0=_ap(t0, s * TS_SET, [(TT, G * NH), (T, T), (1, H)]),
                    in1=_ap(t1, s * TS_SET, [(TT, G * NH), (T, T), (1, H)]))
                ro = H * T + H
                vector.tensor_add(
                    out=_ap(t0, s * TS_SET + ro,
                            [(TT, G * NH), (T, T - H), (1, T - H)]),
                    in0=_ap(t0, s * TS_SET + ro,
                            [(TT, G * NH), (T, T - H), (1, T - H)]),
                    in1=_ap(t1, s * TS_SET + ro,
                            [(TT, G * NH), (T, T - H), (1, T - H)])
                ).then_inc(sems["s_done"], 1)

            def phase_b(n):
                s = n % 2
                vector.wait_ge(sems["e_done"], n + 1)
                if n >= 2:
                    # WAR: res[s] still being read by out-DMA(n-2) (same parity)
                    vector.wait_ge(sems["out0" if s == 0 else "out1"],
                                   16 * (n // 2))
                # row sums over j (i<H reads only j<H), then reciprocal
                vector.tensor_reduce(
                    out=_ap(den, 0, [(NH * T, G), (T, NH), (1, H)]),
                    in_=_ap(t1, s * TS_SET, [(TT, G * NH), (T, H), (1, H)]),
                    axis=mybir.AxisListType.X, op=mybir.AluOpType.add)
                vector.tensor_reduce(
                    out=_ap(den, H, [(NH * T, G), (T, NH), (1, T - H)]),
                    in_=_ap(t1, s * TS_SET + H * T,
                            [(TT, G * NH), (T, T - H), (1, T)]),
                    axis=mybir.AxisListType.X, op=mybir.AluOpType.add)
                vector.reciprocal(
                    out=_ap(rcp, 0, [(1, G * NH * T)]),
                    in_=_ap(den, 0, [(1, G * NH * T)]))
                # PV: pp = e * v (blocked), then o2[g,i,(h,c)] = sum_j pp
                for c in range(HD):
                    for h in range(NH):
                        for bi, (i0b, j0b) in enumerate(BLKS):
                            li = H if i0b == 0 else T - H
                            lj = H if j0b == 0 else T - H
                            mm = vector.tensor_mul(
                                out=_ap(pp, h * TT + i0b * T + j0b,
                                        [(NH * TT, G), (T, li), (1, lj)]),
                                in0=_ap(t1, s * TS_SET + h * TT + i0b * T + j0b,
                                        [(TS_G, G), (T, li), (1, lj)]),
                                in1=_ap(yv, s * YV_SET + YV_W + (h * HD + c) * T
                                        + j0b, [(YV_G, G), (0, li), (1, lj)]))
                            if c == HD - 1 and h == NH - 1 and bi == len(BLKS) - 1:
                                mm.then_inc(sems["b_done"], 1)
                    vector.tensor_reduce(
                        out=_ap(o2, c, [(T * D, G), (HD, NH), (D, H)]),
                        in_=_ap(pp, 0, [(TT, G * NH), (T, H), (1, H)]),
                        axis=mybir.AxisListType.X, op=mybir.AluOpType.add)
                    vector.tensor_reduce(
                        out=_ap(o2, c + H * D,
                                [(T * D, G), (HD, NH), (D, T - H)]),
                        in_=_ap(pp, H * T,
                                [(TT, G * NH), (T, T - H), (1, T)]),
                        axis=mybir.AxisListType.X, op=mybir.AluOpType.add)
                # normalize: o2 *= rcp (broadcast over c)
                for g in range(G):
                    vector.tensor_mul(
                        out=_ap(o2, g * T * D, [(D, T), (HD, NH), (1, HD)]),
                        in0=_ap(o2, g * T * D, [(D, T), (HD, NH), (1, HD)]),
                        in1=_ap(rcp, g * NH * T, [(1, T), (T, NH), (0, HD)]))
                # output projection: res[g,t,dm] = sum_e o2[g,t,e]*WoM[dm,e]
                for g in range(G):
                    vector.tensor_mul(
                        out=_ap(prod, g * D * T * D,
                                [(T * D, D), (D, T), (1, D)]),
                        in0=_ap(o2, g * T * D, [(0, D), (D, T), (1, D)]),
                        in1=_ap(wsb, OFF_WO, [(D, D), (0, T), (1, D)]))
                vector.tensor_reduce(
                    out=_ap(res, s * XIN_SET, [(T * D, G), (1, D), (D, T)]),
                    in_=_ap(prod, 0, [(D, G * D * T), (1, D)]),
                    axis=mybir.AxisListType.X, op=mybir.AluOpType.add
                ).then_inc(sems["res_done"], 1)

            for n in range(NT):
                phase_a(n)
                if n >= 1:
                    phase_b(n - 1)
            phase_b(NT - 1)

    return nc


def _pack_weights(Wq, Wk, Wv, Wo):
    wts = np.zeros(CLEN, dtype=np.float32)
    scale = 1.0 / math.sqrt(HD)
    A2 = wts[OFF_A2:OFF_A2 + 36].reshape(2, D, POS)
    for h in range(NH):
        A2[0, h * HD:(h + 1) * HD, :] = (Wq[h * HD:(h + 1) * HD, :].T
                                         @ Wk[h * HD:(h + 1) * HD, :]) * scale
        A2[1, h * HD:(h + 1) * HD, :] = Wv[h * HD:(h + 1) * HD, :]
    wts[OFF_WO:OFF_WO + 36] = Wo.reshape(-1)
    mask = np.where(np.tril(np.ones((T, T))) > 0, 0.0, -1e9).astype(np.float32)
    wts[OFF_MASK:OFF_MASK + TT] = mask.reshape(-1)
    return wts


@lru_cache(maxsize=2)
def _cached_kernel(bc, G):
    return build_kernel(bc, G)


def kernel(x, Wq, Wk, Wv, Wo):
    x = np.ascontiguousarray(x, dtype=np.float32)
    B = x.shape[0]
    bc = B // NCORES
    G = 2
    nc = _cached_kernel(bc, G)
    wts = _pack_weights(np.asarray(Wq, dtype=np.float32),
                        np.asarray(Wk, dtype=np.float32),
                        np.asarray(Wv, dtype=np.float32),
                        np.asarray(Wo, dtype=np.float32))
    in_maps = [{"x": x[i * bc:(i + 1) * bc], "wts": wts} for i in range(NCORES)]
    r = run_bass_kernel_spmd(nc, in_maps, core_ids=list(range(NCORES)))
    return np.concatenate([m["out"] for m in r.results], axis=0)


# revision 21
# speedup vs baseline: 1.6413x; 1.0830x over previous
"""Trainium2 Bass kernel for nn_Attn_40046275068166.

Tiny causal MHA over huge batch: x[B=65536, T=34, D=6], 2 heads, head_dim 3.
Strategy: pure data parallelism over 8 cores (batch sharded), batch on the
128 SBUF partitions inside each core. All per-example compute is expressed
as DVE tensor ops with broadcast access patterns; exp runs on the scalar
engine (ACT). Software-pipelined: phase A (projections + scores) of tile n
overlaps ACT exp of tile n-1 and phase B (softmax-normalize + PV + output
projection) of tile n-1. Raw bass (no Tile framework) with explicit
semaphores — this walrus build allows at most one sync-wait per instruction,
so every multi-dependency is expressed as standalone wait ops.

Math identity used to skip separate q/k projections:
  s[b,h,i,j] = q_i . k_j / sqrt(hd) = xp_i^T A_h xp_j,  A_h = Wq_h^T Wk_h/sqrt(hd)
so only y = A_h xp (per j) and v = Wv xt are projected, and s = xp_i . y_j.
Causal mask applied additively (-1e9) before exp.
"""

import math
from contextlib import ExitStack
from functools import lru_cache

import numpy as np

import concourse.bass as bass
from concourse import mybir
from concourse.bass_utils import run_bass_kernel_spmd

NCORES = 8
T = 34
D = 6
NH = 2
HD = 3
POS = 3
TT = T * T          # 1156
STT = NH * TT       # 2312 score elems per example
P = 128

F32 = mybir.dt.float32

# constants vector layout (element offsets)
OFF_A2 = 0          # [2][6][3]  w=0: y-proj weights, w=1: v-proj weights
OFF_WO = 36         # [6][6]     WoM[dm][e]
OFF_MASK = 72       # [1156]     additive causal mask (0 / -1e9)
CLEN = 72 + TT


def _ap(t, off, dims):
    """AP on SBUF tensor t: explicit free dims [(stride, count), ...]."""
    p0 = t[:].ap[0]
    return bass.AP(tensor=t, offset=off, ap=[list(p0)] + [list(d) for d in dims])


def build_kernel(bc, G):
    """bc: per-core batch, G: b-groups of 128 per pipeline tile."""
    assert bc % (P * G) == 0
    NT = bc // (P * G)
    GT = G * T * D          # x elements per partition per set (g,t,d)
    SC = G * STT            # score elems per partition per set

    nc = bass.Bass("TRN2")
    x = nc.dram_tensor("x", [bc, T, D], F32, kind="ExternalInput")
    wts = nc.dram_tensor("wts", [CLEN], F32, kind="ExternalInput")
    out = nc.dram_tensor("out", [bc, T, D], F32, kind="ExternalOutput")

    xr = x[:].rearrange("(n g p) t d -> n p g t d", g=G, p=P)
    outr = out[:].rearrange("(n g p) t d -> n p g t d", g=G, p=P)
    wts_b = bass.AP(tensor=wts, offset=0, ap=[[0, P], [1, CLEN]])

    with ExitStack() as ctx:
        sb = lambda nm, shape: ctx.enter_context(nc.sbuf_tensor(nm, shape, F32))
        wsb = sb("wsb", [P, CLEN])
        xin = sb("xin", [P, 2, G, T, D])
        # [set][w][g][hc][j] — j innermost (stride 1): DVE broadcast reads
        # with non-unit inner strides cost ~1.7x; stride-0/1 run at full rate
        yv = sb("yv", [P, 2, 2, G, D, T])
        pp = sb("pp", [P, G, NH, T, T])   # PV products (dead block stays 0)
        t0 = sb("t0", [P, 2, G, NH, T, T])
        t1 = sb("t1", [P, 2, G, NH, T, T])
        tmp = sb("tmp", [P, T, D])
        den = sb("den", [P, G, NH, T])
        rcp = sb("rcp", [P, G, NH, T])
        o2 = sb("o2", [P, G, T, D])           # [g][t][e=(h,c)]
        prod = sb("prod", [P, G, D, T, D])    # [g][dm][t][e]
        res = sb("res", [P, 2, G, T, D])

        # dma_in/out_sem are parity-split: a DMA's 16 per-engine +1s only
        # certify completion if no OTHER DMA on the same semaphore is in
        # flight (8 engines finishing two DMAs also reads as "16"). With
        # even/odd semaphores plus the xin_done/res_done gating, at most one
        # DMA per semaphore is outstanding when a wait on it passes.
        sem_names = ["dma_in0", "dma_in1", "const", "xin_done", "s_done",
                     "e_done", "b_done", "res_done", "out0", "out1"]
        sems = {k: ctx.enter_context(nc.semaphore(name=k)) for k in sem_names}

        # element strides within a partition
        XIN_SET = G * T * D
        XIN_G = T * D
        YV_SET = 2 * G * T * D
        YV_W = G * T * D
        YV_G = T * D
        TS_SET = G * NH * TT            # t0/t1 set stride
        TS_G = NH * TT
        TS_H = TT

        block = ctx.enter_context(nc.Block())

        @block.gpsimd
        def _(sync):
            # SWDGE (software DGE): exactly one +16 sem increment per
            # dma_start on completion. HWDGE (nc.sync) fans a DMA out over
            # several hardware queues, each of which bumps the semaphore by
            # 16 — a single >=16*(n+1) wait would then fire before the whole
            # transfer has landed.
            def store(k):
                sp = k % 2
                sync.wait_ge(sems["res_done"], k + 1)
                sync.dma_start(
                    out=outr[k],
                    in_=_ap(res, sp * XIN_SET, [(XIN_G, G), (1, T * D)]),
                ).then_inc(sems["out0" if sp == 0 else "out1"], 16)

            sync.dma_start(out=wsb[:], in_=wts_b).then_inc(sems["const"], 16)
            for n in range(NT):
                s = n % 2
                if n >= 2:
                    sync.wait_ge(sems["xin_done"], n - 1)
                sync.dma_start(
                    out=_ap(xin, s * XIN_SET, [(XIN_G, G), (1, T * D)]),
                    in_=xr[n],
                ).then_inc(sems["dma_in0" if s == 0 else "dma_in1"], 16)
                # store lags two tiles so its res_done wait never blocks the
                # next load's descriptor generation in this FIFO
                if n >= 2:
                    store(n - 2)
            store(NT - 2)
            store(NT - 1)
            # quiesce: don't let the program end with the last store in flight
            sync.wait_ge(sems["out0"], 16 * ((NT + 1) // 2))
            sync.wait_ge(sems["out1"], 16 * (NT // 2))

        @block.scalar
        def _(scalar):
            for n in range(NT):
                s = n % 2
                if n >= 2:
                    scalar.wait_ge(sems["b_done"], n - 1)
                scalar.wait_ge(sems["s_done"], n + 1)
                scalar.activation(
                    out=_ap(t1, s * TS_SET, [(1, SC)]),
                    in_=_ap(t0, s * TS_SET, [(1, SC)]),
                    func=mybir.ActivationFunctionType.Exp,
                ).then_inc(sems["e_done"], 1)

        @block.vector
        def _(vector):
            vector.wait_ge(sems["const"], 16)
            # Causal blocks over the TxT score plane (H = T//2):
            #   blk A: i<H,  j<H   (has diagonal -> mask)
            #   blk B: i>=H, j<H   (fully causal -> no mask)
            #   blk C: i>=H, j>=H  (has diagonal -> mask)
            # dead:   i<H,  j>=H   (never computed)
            # t0's dead block is set to -1e9 once (exp -> 0); pp's dead block
            # to 0 once (reduce adds 0). Neither is ever rewritten.
            H = T // 2
            BLKS = [(0, 0), (H, 0), (H, H)]
            for s in range(2):
                vector.memset(
                    _ap(t0, s * TS_SET + H, [(TT, G * NH), (T, H), (1, T - H)]),
                    -1e9)
            vector.memset(_ap(pp, H, [(TT, G * NH), (T, H), (1, T - H)]), 0.0)

            def sadd(dst, dof, i0t, i0o, i1t, i1o):
                """dst[blocks] = in0 + in1 over AB (j<H) and C regions."""
                for (ro, li, lj) in ((0, T, H), (H * T + H, T - H, T - H)):
                    vector.tensor_add(
                        out=_ap(dst, dof + ro, [(TT, G * NH), (T, li), (1, lj)]),
                        in0=_ap(i0t, i0o + ro, [(TT, G * NH), (T, li), (1, lj)]),
                        in1=_ap(i1t, i1o + ro, [(TT, G * NH), (T, li), (1, lj)]))

            def phase_a(n):
                s = n % 2
                vector.wait_ge(sems["dma_in0" if s == 0 else "dma_in1"],
                               16 * (n // 2 + 1))
                # projections: yv[s, w, g, hc, j] = sum_b x[j, base+b]*A2[w,hc,b]
                for w in range(2):
                    for g in range(G):
                        xoff = s * XIN_SET + g * XIN_G + (3 - 3 * w)
                        yoff = s * YV_SET + w * YV_W + g * YV_G
                        for b in range(POS):
                            i0 = _ap(xin, xoff + b, [(0, D), (D, T)])
                            i1 = _ap(wsb, OFF_A2 + w * 18 + b, [(3, D), (0, T)])
                            if b == 0:
                                vector.tensor_mul(
                                    out=_ap(yv, yoff, [(T, D), (1, T)]),
                                    in0=i0, in1=i1)
                            else:
                                vector.tensor_mul(
                                    out=_ap(tmp, 0, [(T, D), (1, T)]),
                                    in0=i0, in1=i1)
                                vector.tensor_add(
                                    out=_ap(yv, yoff, [(1, T * D)]),
                                    in0=_ap(yv, yoff, [(1, T * D)]),
                                    in1=_ap(tmp, 0, [(1, T * D)]))
                # scores: t[g,h,i,j] = sum_a xp[g,i,a] * y[g,(h,a),j], blocked
                for a in range(POS):
                    dst = t0 if a == 0 else t1
                    for h in range(NH):
                        for bi, (i0b, j0b) in enumerate(BLKS):
                            li = H if i0b == 0 else T - H
                            lj = H if j0b == 0 else T - H
                            mm = vector.tensor_mul(
                                out=_ap(dst, s * TS_SET + h * TT + i0b * T + j0b,
                                        [(TS_G, G), (T, li), (1, lj)]),
                                in0=_ap(xin, s * XIN_SET + 3 + a + i0b * D,
                                        [(XIN_G, G), (D, li), (0, lj)]),
                                in1=_ap(yv, s * YV_SET + (h * HD + a) * T + j0b,
                                        [(YV_G, G), (0, li), (1, lj)]))
                            if a == POS - 1 and h == NH - 1 and bi == len(BLKS) - 1:
                                mm.then_inc(sems["xin_done"], 1)
                    if a == 1:
                        sadd(t0, s * TS_SET, t0, s * TS_SET, t1, s * TS_SET)
                # t1 += mask on diagonal blocks A and C; then t0 += t1
                for ro in (0, H * T + H):
                    vector.tensor_add(
                        out=_ap(t1, s * TS_SET + ro,
                                [(TT, G * NH), (T, H), (1, H)]),
                        in0=_ap(t1, s * TS_SET + ro,
                                [(TT, G * NH), (T, H), (1, H)]),
                        in1=_ap(wsb, OFF_MASK + ro,
                                [(0, G * NH), (T, H), (1, H)]))
                vector.tensor_add(
                    out=_ap(t0, s * TS_SET, [(TT, G * NH), (T, T), (1, H)]),
                    in0=_ap(t0, s * TS_SET, [(TT, G * NH), (T, T), (1, H)]),
                    in1=_ap(t1, s * TS_SET, [(TT, G * NH), (T, T), (1, H)]))
                ro = H * T + H
                vector.tensor_add(
                    out=_ap(t0, s * TS_SET + ro,
                            [(TT, G * NH), (T, T - H), (1, T - H)]),
                    in0=_ap(t0, s * TS_SET + ro,
                            [(TT, G * NH), (T, T - H), (1, T - H)]),
                    in1=_ap(t1, s * TS_SET + ro,
                            [(TT, G * NH), (T, T - H), (1, T - H)])
                ).then_inc(sems["s_done"], 1)

            def phase_b(n):
                s = n % 2
                vector.wait_ge(sems["e_done"], n + 1)
                if n >= 2:
                    # WAR: res[s] still being read by out-DMA(n-2) (same parity)
                    vector.wait_ge(sems["out0" if s == 0 else "out1"],
                                   16 * (n // 2))
                # row sums over j (i<H reads only j<H), then reciprocal
                vector.tensor_reduce(
                    out=_ap(den, 0, [(NH * T, G), (T, NH), (1, H)]),
                    in_=_ap(t1, s * TS_SET, [(TT, G * NH), (T, H), (1, H)]),
                    axis=mybir.AxisListType.X, op=mybir.AluOpType.add)
                vector.tensor_reduce(
                    out=_ap(den, H, [(NH * T, G), (T, NH), (1, T - H)]),
                    in_=_ap(t1, s * TS_SET + H * T,
                            [(TT, G * NH), (T, T - H), (1, T)]),
                    axis=mybir.AxisListType.X, op=mybir.AluOpType.add)
                vector.reciprocal(
                    out=_ap(rcp, 0, [(1, G * NH * T)]),
                    in_=_ap(den, 0, [(1, G * NH * T)]))
                # PV: pp = e * v (blocked), then o2[g,i,(h,c)] = sum_j pp
                for c in range(HD):
                    for h in range(NH):
                        for bi, (i0b, j0b) in enumerate(BLKS):
                            li = H if i0b == 0 else T - H
                            lj = H if j0b == 0 else T - H
                            mm = vector.tensor_mul(
                                out=_ap(pp, h * TT + i0b * T + j0b,
                                        [(NH * TT, G), (T, li), (1, lj)]),
                                in0=_ap(t1, s * TS_SET + h * TT + i0b * T + j0b,
                                        [(TS_G, G), (T, li), (1, lj)]),
                                in1=_ap(yv, s * YV_SET + YV_W + (h * HD + c) * T
                                        + j0b, [(YV_G, G), (0, li), (1, lj)]))
                            if c == HD - 1 and h == NH - 1 and bi == len(BLKS) - 1:
                                mm.then_inc(sems["b_done"], 1)
                    vector.tensor_reduce(
                        out=_ap(o2, c, [(T * D, G), (HD, NH), (D, H)]),
                        in_=_ap(pp, 0, [(TT, G * NH), (T, H), (1, H)]),
                        axis=mybir.AxisListType.X, op=mybir.AluOpType.add)
                    vector.tensor_reduce(
                        out=_ap(o2, c + H * D,
                                [(T * D, G), (HD, NH), (D, T - H)]),
                        in_=_ap(pp, H * T,
                                [(TT, G * NH), (T, T - H), (1, T)]),
                        axis=mybir.AxisListType.X, op=mybir.AluOpType.add)
                # normalize: o2 *= rcp (broadcast over c)
                for g in range(G):
                    vector.tensor_mul(
                        out=_ap(o2, g * T * D, [(D, T), (HD, NH), (1, HD)]),
                        in0=_ap(o2, g * T * D, [(D, T), (HD, NH), (1, HD)]),
                        in1=_ap(rcp, g * NH * T, [(1, T), (T, NH), (0, HD)]))
                # output projection: res[g,t,dm] = sum_e o2[g,t,e]*WoM[dm,e]
                for g in range(G):
                    vector.tensor_mul(
                        out=_ap(prod, g * D * T * D,
                                [(T * D, D), (D, T), (1, D)]),
                        in0=_ap(o2, g * T * D, [(0, D), (D, T), (1, D)]),
                        in1=_ap(wsb, OFF_WO, [(D, D), (0, T), (1, D)]))
                vector.tensor_reduce(
                    out=_ap(res, s * XIN_SET, [(T * D, G), (1, D), (D, T)]),
                    in_=_ap(prod, 0, [(D, G * D * T), (1, D)]),
                    axis=mybir.AxisListType.X, op=mybir.AluOpType.add
                ).then_inc(sems["res_done"], 1)

            for n in range(NT):
                phase_a(n)
                if n >= 1:
                    phase_b(n - 1)
            phase_b(NT - 1)

    return nc


def _pack_weights(Wq, Wk, Wv, Wo):
    wts = np.zeros(CLEN, dtype=np.float32)
    scale = 1.0 / math.sqrt(HD)
    A2 = wts[OFF_A2:OFF_A2 + 36].reshape(2, D, POS)
    for h in range(NH):
        A2[0, h * HD:(h + 1) * HD, :] = (Wq[h * HD:(h + 1) * HD, :].T
                                         @ Wk[h * HD:(h + 1) * HD, :]) * scale
        A2[1, h * HD:(h + 1) * HD, :] = Wv[h * HD:(h + 1) * HD, :]
    wts[OFF_WO:OFF_WO + 36] = Wo.reshape(-1)
    mask = np.where(np.tril(np.ones((T, T))) > 0, 0.0, -1e9).astype(np.float32)
    wts[OFF_MASK:OFF_MASK + TT] = mask.reshape(-1)
    return wts


@lru_cache(maxsize=2)
def _cached_kernel(bc, G):
    return build_kernel(bc, G)


def kernel(x, Wq, Wk, Wv, Wo):
    x = np.ascontiguousarray(x, dtype=np.float32)
    B = x.shape[0]
    bc = B // NCORES
    G = 2
    nc = _cached_kernel(bc, G)
    wts = _pack_weights(np.asarray(Wq, dtype=np.float32),
                        np.asarray(Wk, dtype=np.float32),
                        np.asarray(Wv, dtype=np.float32),
                        np.asarray(Wo, dtype=np.float32))
    in_maps = [{"x": x[i * bc:(i + 1) * bc], "wts": wts} for i in range(NCORES)]
    r = run_bass_kernel_spmd(nc, in_maps, core_ids=list(range(NCORES)))
    return np.concatenate([m["out"] for m in r.results], axis=0)
